# revision 33
# baseline (speedup 1.0000x reference)
"""Multi-head causal+padded attention on 8 TRN2 NeuronCores.

Data-parallel over batch (8 batches -> 8 cores, no collectives).
Transposed layout per core (no PE transposes of the attention matrix):

  QT[h] = (q Wq^T)^T slice  [e=128, tq]   KT[h] likewise   (f16 matmuls)
  V2[p] = (k Wv^T) row-chunk pairs, fp8e4 interleaved [tk=128, 2, he]
  S^T   = KT^T-block @ QT   [key-part, query-free]  (f16, causal diag via
          one batched bf16-free tri matmul injected into PSUM)
  A^T   = exp(s*S^T + bias_k)  -> fp8e4, key-pad mask folded into the
          per-partition exp bias (masked keys underflow to exact 0),
          one merged activation per head-PAIR per key chunk
  sums  = ones2 (fp8 DoubleRow) @ A^T-pairs  + bf16 case fix matmul
  outT  = V2 (fp8 DoubleRow) @ A^T-pairs
  attnT = outT * recip(sums)   (DVE recip; mult split DVE/Pool)
  out^T = sum_h WuT[h]^T @ attnT[h] + w2@brow + bu   (per query half,
          half-0 projection+store overlapped with half-1 attention)

Degenerate softmax rows are fixed exactly via the caserow 1e30 sum bias
plus a host-computed rank-2 mean-of-V correction (w2 @ brow).
"""

import numpy as np
import ml_dtypes

import concourse.bacc as bacc
import concourse.mybir as mybir
import concourse.tile as tile
from concourse.bass_utils import run_bass_kernel_spmd

F32 = mybir.dt.float32
F32R = mybir.dt.float32r
BF16 = mybir.dt.bfloat16
F16 = mybir.dt.float16
FP8 = mybir.dt.float8e4

B, TQ, TK, E, H = 8, 1024, 1024, 128, 8
HE = H * E
SCALE = float(E) ** -0.5
TRI_NEG = -60000.0
MASK_BIAS = -30.0  # exp(-30) underflows fp8e4 to exact 0

DR = mybir.MatmulPerfMode.DoubleRow


def _build():
    nc = bacc.Bacc("TRN2", target_bir_lowering=False, debug=False)
    dp = nc.declare_dram_parameter
    d_qT = dp("qT", [E, TQ], F16, isOutput=False)
    d_kT = dp("kT", [E, TK], F16, isOutput=False)
    d_mall = dp("mall", [E, HE], F16, isOutput=False)
    d_wvT = dp("wvT", [E, HE], F16, isOutput=False)
    d_wuT = dp("wuT", [HE, E], F16, isOutput=False)
    d_tri8 = dp("tri8", [64, 512], FP8, isOutput=False)
    d_idb8 = dp("identb8", [64, 256], FP8, isOutput=False)
    d_bias = dp("biasmk", [128, 8], F32, isOutput=False)
    d_case = dp("caserow", [1, TQ], BF16, isOutput=False)
    d_ones1 = dp("onesk1", [1, 128], BF16, isOutput=False)
    d_ones2 = dp("ones2", [128, 256], FP8, isOutput=False)
    d_onesf = dp("onesf", [128, 128], F16, isOutput=False)
    d_brow = dp("brows", [2, TQ], F32R, isOutput=False)
    d_w2 = dp("w2", [2, E], F32R, isOutput=False)
    d_bu = dp("bu", [E, 1], F32, isOutput=False)
    d_out = dp("out", [E, TQ], F16, isOutput=True)

    Exp = mybir.ActivationFunctionType.Exp
    Ident = mybir.ActivationFunctionType.Identity
    mult = mybir.AluOpType.mult
    mm = nc.tensor.matmul

    with tile.TileContext(nc) as tc:
        with (
            tc.tile_pool(name="const", bufs=1) as cp,
            tc.tile_pool(name="persist", bufs=1) as pp,
        ):
            # ---- input DMAs: big tensors split across engine queues ----
            qTs = cp.tile([E, TQ], F16, tag="qTs", name="qTs")
            kTs = cp.tile([E, TK], F16, tag="kTs", name="kTs")
            mall = cp.tile([E, HE], F16, tag="mall", name="mall")
            wv = cp.tile([E, HE], F16, tag="wv", name="wv")
            nc.gpsimd.dma_start(out=mall[:], in_=d_mall[:])
            nc.scalar.dma_start(out=qTs[:, 0:512], in_=d_qT[:, 0:512])
            nc.sync.dma_start(out=qTs[:, 512:TQ], in_=d_qT[:, 512:TQ])
            nc.scalar.dma_start(out=kTs[:, 0:512], in_=d_kT[:, 0:512])
            nc.sync.dma_start(out=kTs[:, 512:TK], in_=d_kT[:, 512:TK])
            nc.gpsimd.dma_start(out=wv[:], in_=d_wvT[:])

            # ---- constants (gpsimd queue: cheapest trigger) ----
            wuall = cp.tile([128, H, 128], F16, tag="wuall", name="wuall")
            nc.gpsimd.dma_start(
                out=wuall[:],
                in_=d_wuT.rearrange("(c p) e -> p c e", p=128),
            )
            tri8 = cp.tile([64, 2, 256], FP8, tag="tri8", name="tri8")
            nc.gpsimd.dma_start(out=tri8[:].rearrange("p a b -> p (a b)"),
                                in_=d_tri8[:])
            idb8 = cp.tile([64, 2, 128], FP8, tag="idb8", name="idb8")
            nc.gpsimd.dma_start(out=idb8[:].rearrange("p a b -> p (a b)"),
                                in_=d_idb8[:])
            biasmk = cp.tile([128, 8], F32, tag="biasmk", name="biasmk")
            nc.gpsimd.dma_start(out=biasmk[:], in_=d_bias[:])
            case = cp.tile([1, TQ], BF16, tag="case", name="case")
            nc.gpsimd.dma_start(out=case[:], in_=d_case[:])
            ones1 = cp.tile([1, 128], BF16, tag="ones1", name="ones1")
            nc.gpsimd.dma_start(out=ones1[:], in_=d_ones1[:])
            ones2 = cp.tile([128, 2, 128], FP8, tag="ones2", name="ones2")
            nc.gpsimd.dma_start(
                out=ones2[:].rearrange("p a b -> p (a b)"), in_=d_ones2[:]
            )
            onesf = cp.tile([128, 128], F16, tag="onesf", name="onesf")
            nc.gpsimd.dma_start(out=onesf[:], in_=d_onesf[:])
            brow = cp.tile([2, TQ], F32R, tag="brow", name="brow")
            nc.gpsimd.dma_start(out=brow[:], in_=d_brow[:])
            w2 = cp.tile([2, 128], F32R, tag="w2", name="w2")
            nc.gpsimd.dma_start(out=w2[:], in_=d_w2[:])
            bu = cp.tile([E, 1], F32, tag="bu", name="bu")
            nc.gpsimd.dma_start(out=bu[:], in_=d_bu[:])

            # ---- persistent activations ----
            # QT[h] holds G_h = (Wq_h^T Wk_h)^T @ qT; scores then contract
            # G_h against the raw masked kT input (no K projection needed)
            QT = [pp.tile([128, TQ], F16, tag=f"QT{h}", name=f"QT{h}") for h in range(H)]
            # V2[p]: key chunks (2p, 2p+1) interleaved for DoubleRow (half 1);
            # Vf[kc]: f16 copies of chunks 0-3 for the f16 half-0 consume
            V2 = [pp.tile([128, 2, HE], FP8, tag=f"V2{p}", name=f"V2{p}") for p in range(4)]
            Vf = [pp.tile([128, HE], F16, tag=f"Vf{kc}", name=f"Vf{kc}") for kc in range(4)]
            # attnT2[hp]: normalized attention outputs, head-pair interleaved
            # in fp8 so the final projection runs as DoubleRow matmuls
            attnT2 = [
                pp.tile([128, 2, TQ], F16, tag=f"attnT2{hp}", name=f"attnT2{hp}")
                for hp in range(4)
            ]
            outsb = pp.tile([E, TQ], F16, tag="outsb", name="outsb")

            # evac engine rotation: PSUM readable only by ACT/DVE; ACT gets a
            # few early ones (it idles in phase 1), DVE the rest
            _act_slots = {0, 2, 4, 6, 8, 10, 12, 14}
            evac_engines = [
                nc.scalar if i in _act_slots else nc.vector for i in range(16)
            ]
            n_evac = 0

            def evac(dst, src):
                nonlocal n_evac
                eng = evac_engines[n_evac % len(evac_engines)]
                if eng is nc.scalar:
                    eng.copy(dst, src)
                else:
                    eng.tensor_copy(dst, src)
                n_evac += 1

            # ---- phase 1: projections (f16 matmuls, 1 cyc/col) ----
            # bufs=4: enough PSUM depth that the projection matmuls run
            # gap-free and the PE ramps to its full p-state
            with tc.tile_pool(name="ppsum", bufs=4, space="PSUM") as jps:
                def proj_g(h):
                    ps = jps.tile([128, TQ], F32, tag="pps", name=f"psg{h}")
                    for a, b in ((0, 512), (512, TQ)):
                        mm(ps[:, a:b], mall[:, h * 128 : (h + 1) * 128],
                           qTs[:, a:b], start=True, stop=True)
                    evac(QT[h][:], ps[:])

                def proj_v(kc):
                    ps = jps.tile([128, HE], F32, tag="pps", name=f"psv{kc}")
                    for a, b in ((0, 512), (512, HE)):
                        mm(ps[:, a:b], kTs[:, kc * 128 : (kc + 1) * 128],
                           wv[:, a:b], start=True, stop=True)
                    if kc < 4:
                        # f16 for the half-0 consume; fp8 repack on the (idle)
                        # gpsimd engine, SBUF->SBUF
                        evac(Vf[kc][:], ps[:])
                        nc.gpsimd.tensor_copy(V2[kc // 2][:, kc % 2, :], Vf[kc][:])
                    else:
                        evac(V2[kc // 2][:, kc % 2, :], ps[:])

                # first unit (hp0, half0) needs G h0,h1 and V chunks 0-3
                proj_g(0); proj_g(1)
                for kc in range(4):
                    proj_v(kc)
                proj_g(2); proj_g(3)
                for kc in range(4, 8):
                    proj_v(kc)
                for h in range(4, 8):
                    proj_g(h)

            # ---- phase 3+4: attention, unit = (head pair, query half) ----
            with (
                tc.tile_pool(name="accps", bufs=1, space="PSUM") as ap_,
                tc.tile_pool(name="stps", bufs=2, space="PSUM") as sp,
                tc.tile_pool(name="atp", bufs=6) as atp,
                tc.tile_pool(name="ssp", bufs=4) as ssp,
            ):
                # half-major order: all head pairs for half 0, then half 1
                units = [(hp, half) for half in (0, 1) for hp in range(4)]
                state = {}
                n_mult = 0

                def emit_epilogue(u):
                    nonlocal n_mult
                    hp, half = units[u]
                    q0 = half * 512
                    for h in (0, 1):
                        sum_ps, out_ps = state[u][h]
                        rb = ssp.tile([128, 512], F32, tag=f"rb{h}", name=f"rb{u}_{h}")
                        nc.vector.reciprocal_approx_fast(out=rb[:], in_=sum_ps[:])
                        nc.vector.tensor_tensor(
                            out=attnT2[hp][:, h, q0 : q0 + 512], in0=out_ps[:],
                            in1=rb[:], op=mult,
                        )
                        n_mult += 1

                def emit_phase4(half):
                    q0 = half * 512
                    fin = ap_.tile([128, 512], F32, tag="sum0", name=f"fin{half}")
                    for gh in range(H):
                        mm(fin[:], wuall[:, gh, :],
                           attnT2[gh // 2][:, gh % 2, q0 : q0 + 512],
                           start=(gh == 0), stop=False)
                    mm(fin[:], w2[:], brow[:, q0 : q0 + 512],
                       start=False, stop=True)
                    nc.scalar.activation(
                        out=outsb[:, q0 : q0 + 512], in_=fin[:], func=Ident,
                        bias=bu[:, 0:1], scale=1.0,
                    )
                    engs = [nc.gpsimd, nc.scalar, nc.sync, nc.gpsimd]
                    for i in range(4):
                        a = q0 + i * 128
                        engs[i].dma_start(out=d_out[:, a : a + 128],
                                          in_=outsb[:, a : a + 128])

                for u, (hp, half) in enumerate(units):
                    q0 = half * 512
                    kcs = list(range(4) if half == 0 else range(8))
                    npair = len(kcs) // 2
                    sum_ps = [None, None]
                    out_ps = [None, None]
                    ats = {}

                    def alloc_acc():
                        for h in (0, 1):
                            sum_ps[h] = ap_.tile([128, 512], F32, tag=f"sum{h}",
                                                 name=f"sum{u}_{h}")
                            out_ps[h] = ap_.tile([128, 512], F32, tag=f"out{h}",
                                                 name=f"out{u}_{h}")
                        state[u] = [(sum_ps[h], out_ps[h]) for h in (0, 1)]

                    def consume0(kc, last):
                        # f16 path (query half 0): per-chunk, no DoubleRow.
                        # sum bank was zero-based by the case matmul already.
                        r0 = kc * 128
                        at = ats[kc]
                        for h in (0, 1):
                            gh = 2 * hp + h
                            mm(sum_ps[h][:, r0:512], onesf[:],
                               at[:, h, r0:512], start=False, stop=last)
                            mm(out_ps[h][:, r0:512],
                               Vf[kc][:, gh * 128 : (gh + 1) * 128],
                               at[:, h, r0:512], start=(kc == 0), stop=last)

                    def consume(p, last):
                        # fp8 DoubleRow path (query half 1): per chunk pair
                        a, b = 2 * p, 2 * p + 1
                        r0a = max(a * 128 - q0, 0)
                        r0b = max(b * 128 - q0, 0)
                        at = ats[p]
                        first = p == 0
                        fringe = r0b > r0a
                        # start=True marks the whole 2KB PSUM bank pending-zero,
                        # so it must appear exactly once per bank per group
                        for h in (0, 1):
                            gh = 2 * hp + h
                            vsl = V2[p][:, :, gh * 128 : (gh + 1) * 128]
                            if fringe:
                                mm(sum_ps[h][:, r0a:r0b], ones2[:, 0, :],
                                   at[:, h, 0, r0a:r0b], start=False, stop=False)
                                mm(out_ps[h][:, r0a:r0b], V2[p][:, 0, gh * 128 : (gh + 1) * 128],
                                   at[:, h, 0, r0a:r0b], start=first, stop=False)
                            mm(sum_ps[h][:, r0b:512], ones2[:],
                               at[:, h, :, r0b:512], start=False,
                               stop=last, perf_mode=DR)
                            mm(out_ps[h][:, r0b:512], vsl,
                               at[:, h, :, r0b:512], start=first and not fringe,
                               stop=last, perf_mode=DR)

                    for kc in kcs:
                        r0 = max(kc * 128 - q0, 0)
                        diag = q0 <= kc * 128 < q0 + 512
                        p = kc // 2
                        st = sp.tile([128, 2, 512], F32, tag="st", name=f"st{u}_{kc}")
                        for h in (0, 1):
                            gh = 2 * hp + h
                            mm(st[:, h, r0:512],
                               kTs[:, kc * 128 : (kc + 1) * 128],
                               QT[gh][:, q0 + r0 : q0 + 512], start=True,
                               stop=not diag)
                        if diag:
                            mm(st[:, :, r0 : r0 + 128], idb8[:], tri8[:],
                               start=False, stop=True, perf_mode=DR)
                        if half == 0:
                            ats[kc] = atp.tile([128, 2, 512], F16, tag="atf",
                                               name=f"atf{u}_{kc}")
                            exp_out = ats[kc][:, :, r0:512]
                        else:
                            if kc % 2 == 0:
                                ats[p] = atp.tile([128, 2, 2, 512], FP8, tag="at8",
                                                  name=f"at8{u}_{p}")
                            exp_out = ats[p][:, :, kc % 2, r0:512]
                        nc.scalar.activation(
                            out=exp_out,
                            in_=st[:, :, r0:512], func=Exp,
                            bias=biasmk[:, kc : kc + 1], scale=SCALE,
                        )
                        if kc == 0 and u == 4:
                            # half-0 output projection; before this unit's acc
                            # allocation so the borrowed sum0 generations match
                            emit_phase4(0)
                        if kc == 1:
                            alloc_acc()
                            # zero-base the sum banks with the case-fix row so
                            # the reciprocal can fire right after the last
                            # consume matmul of this unit
                            for h in (0, 1):
                                mm(sum_ps[h][:], ones1[:],
                                   case[:, q0 : q0 + 512], start=True, stop=False)
                        if half == 0:
                            if kc >= 2:
                                consume0(kc - 2, last=False)
                        elif kc % 2 == 1 and p >= 1:
                            consume(p - 1, last=False)
                    if half == 0:
                        consume0(2, last=False)
                        consume0(3, last=True)
                    else:
                        consume(npair - 1, last=True)
                    emit_epilogue(u)
                emit_phase4(1)

    nc.compile()
    return nc


_NC = None


def _get_nc():
    global _NC
    if _NC is None:
        _NC = _build()
    return _NC


def _host_prep(q, k, mask_q, mask_k, Wq, Wk, Wv, Wu, bu):
    tri = -240.0 * np.tril(np.ones((128, 128), np.float32), -1)
    tri8 = np.concatenate([tri, tri], axis=1).reshape(128, 2, 128)
    eye = np.eye(128, dtype=np.float32)
    shared = {
        "mall": np.concatenate(
            [Wq[h * E : (h + 1) * E].T @ Wk[h * E : (h + 1) * E]
             for h in range(H)], axis=1).astype(np.float16),
        "wvT": np.ascontiguousarray(Wv.T).astype(np.float16),
        "wuT": np.ascontiguousarray(Wu.T).astype(np.float16),
        "tri8": np.concatenate(
            [np.concatenate([tri[0:64], tri[0:64]], axis=1),
             np.concatenate([tri[64:128], tri[64:128]], axis=1)],
            axis=1).astype(ml_dtypes.float8_e4m3)[0:64],
        "identb8": np.concatenate([eye[0:64], eye[64:128]], axis=1).astype(
            ml_dtypes.float8_e4m3),
        "onesk1": np.ones((1, 128), np.float32).astype(ml_dtypes.bfloat16),
        "ones2": np.ones((128, 256), np.float32).astype(ml_dtypes.float8_e4m3),
        "onesf": np.ones((128, 128), np.float16),
        "bu": np.ascontiguousarray(bu[:, None]).astype(np.float32),
    }
    WuWv = (Wu @ Wv).astype(np.float32)
    in_maps = []
    for b in range(B):
        mq = mask_q[b, :, 0].astype(np.float32)
        mk = mask_k[b, :, 0].astype(np.float32)
        c01 = (np.cumsum(mk) >= 1.0).astype(np.float32)
        caseA = mq * c01
        b1 = mq * (1.0 - c01)
        b2 = 1.0 - mq
        s1m = 1.0 - mk
        denom = max(float(s1m.sum()), 1.0)
        wvecs = np.stack([s1m / denom, np.full(TK, 1.0 / TK, np.float32)], axis=1)
        w2 = (wvecs.T.astype(np.float32) @ k[b]) @ WuWv.T
        m = dict(shared)
        m["qT"] = np.ascontiguousarray(q[b].T).astype(np.float16)
        m["kT"] = np.ascontiguousarray((k[b] * mk[:, None]).T).astype(np.float16)
        m["biasmk"] = np.ascontiguousarray(
            np.where(mk.reshape(8, 128).T == 1.0, 0.0, MASK_BIAS)
        ).astype(np.float32)
        m["caserow"] = ((1.0 - caseA) * 1.0e30)[None, :].astype(ml_dtypes.bfloat16)
        m["brows"] = np.stack([b1, b2]).astype(np.float32)
        m["w2"] = np.ascontiguousarray(w2.astype(np.float32))
        in_maps.append(m)
    return in_maps


def kernel(q, k, mask_q, mask_k, Wq, Wk, Wv, Wu, bu):
    nc = _get_nc()
    in_maps = _host_prep(q, k, mask_q, mask_k, Wq, Wk, Wv, Wu, bu)
    res = run_bass_kernel_spmd(nc, in_maps, list(range(B)))
    out = np.stack([np.ascontiguousarray(res.results[b]["out"].T) for b in range(B)])
    return out.astype(np.float32)
